# revision 45
# baseline (speedup 1.0000x reference)
"""Bahdanau-attention GRU cell fused Trainium2 kernel.

Sharding: data-parallel over batch across 8 NeuronCores (4 batch rows per
core, weights replicated, no collectives).

Math per core (b=4 local batch rows, T=2048, F=U=512):
  annotations stream in natural [t, f] layout, SWDGE-cast fp32->fp8e4 on
  load; the fp8 tiles feed both the PE transposes and (as the mixed-dtype
  moving operand) the context matmuls, so no second cast pass exists.
  PE transposes tiles to annT [f, t] in fp8; HW fp8 transpose-mode writes
  PSUM at 16-bit granularity (value, garbage byte), so the psum->sbuf
  copies move uint16 pairs at DVE 2x rate and the pre matmuls read a
  stride-2 fp8 view.
  pre^T[u,t] = Ua^T annT via fp8 DoubleRow matmuls (contraction passes
  halved); tanh fused on ScalarE with per-partition bias qT = Wa h +
  biases, output fp8.
  scores = Va . tanh(pre) via fp8 DoubleRow with Va replicated across
  the 128 output partitions; exp on ScalarE (no max-sub; |scores| <~ 20)
  with accum_out collecting the softmax normalizer Z.
  Pipeline depth: transposes/copies for pair j, pre+tanh for pair j-1,
  scores/exp for pair j-2, context matmuls for pair j-3 -- the PE never
  waits on the DVE copy chain nor on ScalarE's tanh/exp.
  context: the p row is PE-transposed to columns, then c accumulates via
  PE matmuls (stationary = p column chunk, bf16; moving = natural fp8
  annotation tile) across 4 concurrent tile_position column-groups whose
  partials are summed in the tail.
  GRU gates: one PSUM accumulation of x@K + c@AK + h@RK[:,:2U] + biases,
  hard-sigmoid/tanh epilogue, h_new = z*h + (1-z)*hh.
  GRU/attention weights are loaded and cast once (rep 0) and stay
  SBUF-resident across reps; x/h-dependent prep re-runs every rep.
"""

import sys

if "/opt/trn_rl_repo" not in sys.path:
    sys.path.insert(0, "/opt/trn_rl_repo")

import numpy as np

import concourse.bass as bass
import concourse.tile as tile
from concourse import bacc, bass_utils, mybir
from concourse.masks import make_identity

F32 = mybir.dt.float32
BF16 = mybir.dt.bfloat16
FP8 = mybir.dt.float8e4
U16 = mybir.dt.uint16
AF = mybir.ActivationFunctionType
ALU = mybir.AluOpType
DR = mybir.MatmulPerfMode.DoubleRow

B, T, F, U = 32, 2048, 512, 512
NCORES = 8
BL = B // NCORES          # 4 local batch rows
TT = 512                  # T-tile (free dim of matmuls)
NTT = T // TT             # 4
NS = TT // 128            # 4 t-subtiles per T-tile
NFB = F // 128            # 4 f blocks
NUB = U // 128            # 4 u blocks
U3 = 3 * U


def build(reps=1, fp8_scores=True, bufs=None):
    bufs = dict(
        dict(annio=6, aT=3, tanh=3, prow=2, pcol=3, tp=2, pre=2, c=1),
        **(bufs or {}),
    )
    nc = bacc.Bacc("TRN2", target_bir_lowering=False, debug=False,
                   dynamic_dma_scratch_size=32768)

    def din(name, shape):
        return nc.dram_tensor(name, shape, F32, kind="ExternalInput").ap()

    d_x = din("x", [BL, F])
    d_h = din("h", [BL, U])
    d_ann = din("annotations", [BL, T, F])
    d_k = din("kernel", [F, U3])
    d_rk = din("recurrent_kernel", [U, U3])
    d_ak = din("attention_kernel", [F, U3])
    d_wa = din("Wa", [U, U])
    d_ua = din("Ua", [F, U])
    d_va = din("Va", [U])
    d_bias = din("bias", [U3])
    d_abias = din("attention_bias", [U3])
    d_wab = din("Wa_bias", [U])
    d_uab = din("Ua_bias", [U])
    d_out = nc.dram_tensor("h_new", [BL, U], F32, kind="ExternalOutput").ap()

    with tile.TileContext(nc) as tc:
        with (
            tc.tile_pool(name="const", bufs=1) as const,
            tc.tile_pool(name="state", bufs=2) as state,
            tc.tile_pool(name="tail", bufs=1) as tail,
            tc.tile_pool(name="annio", bufs=bufs["annio"]) as annio,
            tc.tile_pool(name="aT_p", bufs=bufs["aT"]) as aT_p,
            tc.tile_pool(name="tanh_p", bufs=bufs["tanh"]) as tanh_p,
            tc.tile_pool(name="prow_p", bufs=bufs["prow"]) as prow_p,
            tc.tile_pool(name="pcol_p", bufs=bufs["pcol"]) as pcol_p,
            tc.tile_pool(name="ps_tp", bufs=bufs["tp"], space="PSUM") as ps_tp,
            tc.tile_pool(name="ps_pre", bufs=bufs["pre"], space="PSUM") as ps_pre,
            tc.tile_pool(name="ps_pc", bufs=1, space="PSUM") as ps_pc,
            tc.tile_pool(name="ps_c", bufs=bufs["c"], space="PSUM") as ps_c,
        ):
            # ---------------- constants / weights ----------------
            ident = const.tile([128, 128], BF16)
            make_identity(nc, ident[:])
            ident8 = const.tile([128, 128], FP8)
            make_identity(nc, ident8[:])

            ones4 = const.tile([1, BL], BF16)
            nc.vector.memset(ones4[:], 1.0)
            ident_f1 = const.tile([1, 1], F32)
            nc.vector.memset(ident_f1[:], 1.0)

            # first annotation tile-pair: issue before any weight DMA so the
            # PE pipeline can start transposing as early as possible (two
            # single-tile DMAs so the first half lands sooner).
            # SWDGE casts fp32->fp8 directly on load: natural-layout tiles
            # are consumed as fp8 by both the transposes (fp8 out) and the
            # context matmuls (fp8 moving, bf16 stationary).
            ann_r = d_ann.rearrange("b (tt s p) f -> b tt s p f", p=128, s=NS)
            a2_first = annio.tile([128, 2, NS, F], FP8, tag="ann2",
                                  name="a2_first")
            nc.gpsimd.dma_start(
                out=a2_first[:, 0], in_=ann_r[0, 0].rearrange("s p f -> p s f")
            )
            nc.gpsimd.dma_start(
                out=a2_first[:, 1], in_=ann_r[0, 1].rearrange("s p f -> p s f")
            )

            # SWDGE casts fp32->fp8 directly on load
            ua8 = const.tile([128, NFB, U], FP8)
            nc.gpsimd.dma_start(
                out=ua8[:], in_=d_ua.rearrange("(fb p) u -> p fb u", p=128)
            )
            wa_sb = const.tile([128, NUB, U], BF16)
            nc.gpsimd.dma_start(
                out=wa_sb[:], in_=d_wa.rearrange("(jb p) u -> p jb u", p=128)
            )

            # small vectors: fast HWDGE fp32 loads + on-chip casts
            def row_load(dram_ap, width, nm):
                t32 = const.tile([1, width], F32, name=nm + "_f32")
                nc.sync.dma_start(out=t32[:], in_=dram_ap)
                t16 = const.tile([1, width], BF16, name=nm)
                nc.vector.tensor_copy(t16[:], t32[:])
                return t16

            va_row = row_load(d_va.rearrange("(a u) -> a u", a=1), U, "va_row")
            wab_row = row_load(d_wab.rearrange("(a u) -> a u", a=1), U, "wab_row")
            uab_row = row_load(d_uab.rearrange("(a u) -> a u", a=1), U, "uab_row")
            bias_row = row_load(d_bias.rearrange("(a u) -> a u", a=1), U3, "bias_row")
            abias_row = row_load(d_abias.rearrange("(a u) -> a u", a=1), U3, "abias_row")

            # GRU weights: loaded once (rep 0), SBUF-resident afterwards
            k_sb = const.tile([128, NFB, U3], BF16)
            rk_sb = const.tile([128, NUB, U3], BF16)
            ak_sb = const.tile([128, NFB, U3], BF16)
            k_r = d_k.rearrange("(fb p) u -> p fb u", p=128)
            rk_r = d_rk.rearrange("(fb p) u -> p fb u", p=128)
            ak_r = d_ak.rearrange("(fb p) u -> p fb u", p=128)
            gru_w_chunks = []
            for fb in range(NFB):
                gru_w_chunks.append((k_sb, k_r, fb))
                gru_w_chunks.append((rk_sb, rk_r, fb))
                gru_w_chunks.append((ak_sb, ak_r, fb))

            # VaT replicated: va_rep[p, ub, j] = Va[ub*128+p] for all j
            va_rep = const.tile([128, NUB, 128], BF16)
            for ub in range(NUB):
                tp = ps_tp.tile([128, 128], BF16, tag="tp")
                nc.tensor.transpose(
                    tp[:, 0:1], va_row[0:1, 128 * ub : 128 * (ub + 1)], ident[0:1, 0:1]
                )
                nc.vector.tensor_copy(
                    va_rep[:, ub, :], tp[:, 0:1].to_broadcast([128, 128])
                )
            if fp8_scores:
                va8 = const.tile([128, NUB, 128], FP8)
                nc.vector.tensor_copy(va8[:], va_rep[:])

            pend_tail = [None]
            for _rep in range(reps):
                # ---- per-rep x/h-dependent prep (emitted inside tile 0 so
                # the PE starts on annotation transposes first) ----
                h_f32 = state.tile([BL, U], F32, tag="h_f32", name=f"h_f32_{_rep}")
                x_bf = state.tile([BL, F], BF16, tag="x_bf", name=f"x_bf_{_rep}")
                h_bf = state.tile([BL, U], BF16, tag="h_bf", name=f"h_bf_{_rep}")
                xT = state.tile([128, NFB, BL], BF16, tag="xT", name=f"xT_{_rep}")
                hT = state.tile([128, NUB, BL], BF16, tag="hT", name=f"hT_{_rep}")
                qT = state.tile([128, NUB, BL], F32, tag="qT", name=f"qT_{_rep}")

                def emit_xhq(r=_rep, h_f32=h_f32, x_bf=x_bf,
                             h_bf=h_bf, xT=xT, hT=hT, qT=qT):
                    nc.gpsimd.dma_start(out=x_bf[:], in_=d_x)
                    nc.sync.dma_start(out=h_f32[:], in_=d_h)
                    nc.vector.tensor_copy(h_bf[:], h_f32[:])
                    for jb in range(NFB):
                        tp = ps_tp.tile([128, 128], BF16, tag="tp",
                                        name=f"tpx{r}_{jb}")
                        nc.tensor.transpose(
                            tp[:, 0:BL], x_bf[0:BL, 128 * jb : 128 * (jb + 1)],
                            ident[0:BL, 0:BL],
                        )
                        nc.any.tensor_copy(xT[:, jb, :], tp[:, 0:BL])
                    for jb in range(NUB):
                        tp = ps_tp.tile([128, 128], BF16, tag="tp",
                                        name=f"tph{r}_{jb}")
                        nc.tensor.transpose(
                            tp[:, 0:BL], h_bf[0:BL, 128 * jb : 128 * (jb + 1)],
                            ident[0:BL, 0:BL],
                        )
                        nc.any.tensor_copy(hT[:, jb, :], tp[:, 0:BL])
                    # qT[u, b] = Wa^T h^T + Wa_bias + Ua_bias
                    for ub in range(NUB):
                        qp = ps_tp.tile([128, TT], F32, tag="tp", name=f"qp{r}_{ub}")
                        for jb in range(NUB):
                            nc.tensor.matmul(
                                qp[:, 0:BL],
                                wa_sb[:, jb, 128 * ub : 128 * (ub + 1)],
                                hT[:, jb, :],
                                start=(jb == 0),
                                stop=False,
                            )
                        nc.tensor.matmul(
                            qp[:, 0:BL],
                            wab_row[0:1, 128 * ub : 128 * (ub + 1)],
                            ones4[:],
                            start=False,
                            stop=False,
                        )
                        nc.tensor.matmul(
                            qp[:, 0:BL],
                            uab_row[0:1, 128 * ub : 128 * (ub + 1)],
                            ones4[:],
                            start=False,
                            stop=True,
                        )
                        nc.any.tensor_copy(qT[:, ub, :], qp[:, 0:BL])

                # per-rep accumulators. Context accumulates into 4 PE
                # column-groups concurrently (tile_position col-tiling):
                # group g owns partitions [32g, 32g+BL) of one PSUM bank
                # and takes chunks k%4==g; the tail sums the 4 partials.
                ztile = tail.tile([1, BL * NTT // 2], F32, name=f"ztile{_rep}",
                                  tag="ztile")
                c_ps = ps_c.tile([128, F], F32, name=f"c_ps{_rep}", tag="c_ps")

                # context-independent GRU gate parts (x@K + h@RK + biases),
                # computed mid-loop and parked in SBUF; re-injected into the
                # gate PSUM accumulation via an identity stationary in the tail
                xh_pre = tail.tile([BL, 3, U], BF16, name=f"xh_pre{_rep}",
                                    tag="xh_pre")

                # x@K + h@RK + biases for the three gate blocks, run in three
                # concurrent PE column-groups (tile_position col-tiling);
                # emitted round-robin so consecutive MMs hit different groups
                def emit_gate_pre(nb, r=_rep, xh_pre=xh_pre):
                    gp = ps_pre.tile([4, TT], F32, tag="pre", name=f"gpre{r}_{nb}")
                    n0 = nb * TT
                    for fb in range(NFB):
                        nc.tensor.matmul(
                            gp[:], xT[:, fb, :], k_sb[:, fb, n0 : n0 + TT],
                            start=(fb == 0), stop=False,
                        )
                    if nb < 2:
                        for ub in range(NUB):
                            nc.tensor.matmul(
                                gp[:], hT[:, ub, :], rk_sb[:, ub, n0 : n0 + TT],
                                start=False, stop=False,
                            )
                    nc.tensor.matmul(
                        gp[:], ones4[:], bias_row[0:1, n0 : n0 + TT],
                        start=False, stop=False,
                    )
                    nc.tensor.matmul(
                        gp[:], ones4[:], abias_row[0:1, n0 : n0 + TT],
                        start=False, stop=True,
                    )
                    nc.vector.tensor_copy(xh_pre[:, nb, :], gp[:])

                # deferred context emission: p-transpose + context matmuls of
                # tile i run during tile i+1 so the PE never waits on exp(i).
                # The stationary is a [128, BL] tile with the p column in
                # slot b and zeros elsewhere, so the [BL, F] PSUM region
                # accumulates row b only (matmul out base partition must be 0).
                def emit_ctx(b, pj, p_row2, a2, r=_rep, c_ps=c_ps):
                    p_colT = ps_pc.tile([128, 2 * NS], F32, tag="pcT",
                                        name=f"pcT{r}_{pj}")
                    for k in range(2 * NS):
                        nc.tensor.transpose(
                            p_colT[:, k : k + 1],
                            p_row2[0:1, 128 * k : 128 * (k + 1)],
                            ident_f1[:],
                        )
                    p_col = pcol_p.tile([128, 2, NS, BL], BF16, tag="pcol",
                                        name=f"pc{r}_{pj}")
                    nc.gpsimd.memset(p_col[:], 0.0)
                    nc.vector.tensor_copy(p_col[:, :, :, b], p_colT[:])
                    for h2 in range(2):
                        for s in range(NS):
                            k = h2 * NS + s
                            g = k % 4
                            nc.tensor.matmul(
                                c_ps[32 * g : 32 * g + BL, :],
                                p_col[:, h2, s, :],
                                a2[:, h2, s, :],
                                start=(pj == 0 and k == g),
                                stop=(pj == BL * NTT // 2 - 1 and k == 4 + g),
                                tile_position=(0, 32 * g),
                            )

                pend_a = []
                pend_b = []
                pend_ctx = [None]

                # pre/tanh stage runs one pair behind the transpose/copy
                # stage (so the PE never waits on the DVE-copy latency
                # chain); scores/exp run one further pair behind so the
                # PE never waits on ScalarE's tanh either.
                def emit_stage_a(b, pj, a_T2, a2, r=_rep):
                    t_T2 = tanh_p.tile([128, NUB, 2, TT], FP8, tag="tanhT",
                                       name=f"t_T{r}_{pj}")
                    for ub in range(NUB):
                        pp2 = ps_pre.tile([128, 2, TT], F32, tag="pre",
                                          name=f"pp{r}_{pj}_{ub}")
                        # q outer so consecutive matmuls share the stationary
                        for q in range(2):
                            for h2 in range(2):
                                nc.tensor.matmul(
                                    pp2[:, h2, :],
                                    ua8[:, 2 * q : 2 * q + 2,
                                        128 * ub : 128 * (ub + 1)],
                                    a_T2[:, 2 * q : 2 * q + 2, h2, :],
                                    start=(q == 0),
                                    stop=(q == 1),
                                    perf_mode=DR,
                                )
                        nc.scalar.activation(
                            t_T2[:, ub, :, :], pp2[:], AF.Tanh,
                            bias=qT[:, ub, b : b + 1],
                        )
                    return t_T2

                def emit_stage_b(b, pj, a_T2, a2, t_T2, r=_rep):
                    # scores for both tiles into one 2-bank psum (replicated
                    # across partitions), then a single fused exp per pair
                    sp2 = ps_pre.tile([128, 2, TT], F32, tag="pre",
                                      name=f"sp{r}_{pj}")
                    for m in range(2):
                        for h2 in range(2):
                            nc.tensor.matmul(
                                sp2[:, h2, :],
                                va8[:, 2 * m : 2 * m + 2, :],
                                t_T2[:, 2 * m : 2 * m + 2, h2, :],
                                start=(m == 0),
                                stop=(m == 1),
                                perf_mode=DR,
                            )

                    # deferred context for the pair before this one
                    if pend_ctx[0] is not None:
                        emit_ctx(*pend_ctx[0])

                    # exp + Z partial (row 0 only; no max-sub needed)
                    p_row2 = prow_p.tile([1, 2 * TT], F32, tag="prow",
                                         name=f"p_row{r}_{pj}")
                    nc.scalar.activation(
                        p_row2[:], sp2[0:1, :, :], AF.Exp,
                        accum_out=ztile[:, pj : pj + 1],
                    )
                    pend_ctx[0] = (b, pj, p_row2, a2)

                # ---------------- main attention loop (tile pairs) ----------
                NPAIR = BL * NTT // 2
                for pj in range(NPAIR):
                    b = (2 * pj) // NTT
                    tt0 = (2 * pj) % NTT
                    if _rep == 0 and pj == 0:
                        a2 = a2_first
                    else:
                        a2 = annio.tile([128, 2, NS, F], FP8, tag="ann2",
                                        name=f"a2_{_rep}_{pj}")
                        nc.gpsimd.dma_start(
                            out=a2[:],
                            in_=ann_r[b, tt0 : tt0 + 2].rearrange(
                                "two s p f -> p two s f"
                            ),
                        )
                    # GRU weight loads interleaved into rep 0 only
                    if _rep == 0 and 2 * pj < len(gru_w_chunks):
                        for ci in (2 * pj, 2 * pj + 1):
                            if ci < len(gru_w_chunks):
                                wsb, wr, fb = gru_w_chunks[ci]
                                nc.gpsimd.dma_start(
                                    out=wsb[:, fb, :], in_=wr[:, fb, :]
                                )

                    # transpose both tiles to [f, t] in fp8. HW fp8
                    # transpose-mode writes with output element step 2
                    # (16-bit granularity), so the PSUM tile carries an
                    # interleave dim; the psum->sbuf copies move the
                    # (value, garbage) byte pairs as uint16 at DVE 2x
                    # rate, and the pre matmuls read a stride-2 fp8 view.
                    a_T2u = aT_p.tile([128, NFB, 2, TT], U16, tag="annT",
                                      name=f"a_T{_rep}_{pj}")
                    a_T2 = a_T2u[:].bitcast(FP8).rearrange(
                        "p fb h (t two) -> p fb h t two", two=2
                    )[:, :, :, :, 0]

                    def emit_transposes(fbs, a2=a2, a_T2u=a_T2u, pj=pj):
                        for fb in fbs:
                            tpp = ps_tp.tile([128, 2, TT, 2], FP8, tag="tp",
                                             name=f"tp{_rep}_{pj}_{fb}")
                            for h2 in range(2):
                                for s in range(NS):
                                    nc.tensor.transpose(
                                        tpp[:, h2,
                                            128 * s : 128 * (s + 1), 0],
                                        a2[:, h2, s,
                                           128 * fb : 128 * (fb + 1)],
                                        ident8[:],
                                    )
                            nc.vector.tensor_copy(
                                a_T2u[:, fb, :, :],
                                tpp[:].bitcast(U16).rearrange(
                                    "p h t one -> p h (t one)"
                                ),
                            )

                    # interleave: pre(j-1) matmuls cover the tp-ring slot
                    # waits between the two transpose halves
                    emit_transposes((0, 1))
                    if pend_a:
                        e = pend_a.pop(0)
                        pend_b.append(e + (emit_stage_a(*e),))
                    emit_transposes((2, 3))

                    if pj == 0:
                        emit_xhq()
                    if pj == 5:
                        emit_gate_pre(1)
                    elif pj == 6:
                        emit_gate_pre(2)
                    elif pj == 7:
                        emit_gate_pre(0)

                    # scores/ctx/exp for pair j-2 (tanh of j-1 still runs)
                    if len(pend_b) >= 2:
                        emit_stage_b(*pend_b.pop(0))
                    pend_a.append((b, pj, a_T2, a2))
                    if pj == 0 and pend_tail[0] is not None:
                        # previous rep's GRU tail overlaps this rep's
                        # pipeline fill
                        pend_tail[0]()
                        pend_tail[0] = None

                while pend_a or pend_b:
                    if pend_a:
                        e = pend_a.pop(0)
                        pend_b.append(e + (emit_stage_a(*e),))
                    emit_stage_b(*pend_b.pop(0))
                emit_ctx(*pend_ctx[0])

                def emit_tail(ztile=ztile, c_ps=c_ps, xh_pre=xh_pre,
                              h_f32=h_f32, _rep=_rep):
                    # ---------------- softmax normalization ----------------
                    zsum = tail.tile([1, BL], F32, name=f"zsum{_rep}", tag="zsum")
                    rz = tail.tile([1, BL], F32, name=f"rz{_rep}", tag="rz")
                    npb = NTT // 2  # Z partials per batch row
                    for b in range(BL):
                        nc.vector.reduce_sum(
                            zsum[0:1, b : b + 1],
                            ztile[0:1, b * npb : (b + 1) * npb],
                            axis=mybir.AxisListType.X,
                        )
                    nc.vector.reciprocal(rz[:], zsum[:])
                    rzT_ps = ps_tp.tile([BL, 1], F32, tag="tp", name=f"rzT{_rep}")
                    nc.tensor.transpose(rzT_ps[:], rz[0:1, :], ident_f1[:])
                    rz4 = tail.tile([BL, 1], F32, name=f"rz4{_rep}", tag="rz4")
                    nc.any.tensor_copy(rz4[:], rzT_ps[:])

                    # sum the 4 col-group partials, normalize + cast, then
                    # transpose for the GRU
                    c_s1 = tail.tile([BL, F], F32, name=f"c_s1{_rep}", tag="c_s1")
                    c_s2 = tail.tile([BL, F], F32, name=f"c_s2{_rep}", tag="c_s2")
                    c_s3 = tail.tile([BL, F], F32, name=f"c_s3{_rep}", tag="c_s3")
                    c_sum = tail.tile([BL, F], F32, name=f"c_sum{_rep}", tag="c_sum")
                    nc.vector.tensor_copy(c_s1[:], c_ps[0:BL, :])
                    nc.vector.tensor_add(c_s2[:], c_s1[:], c_ps[32 : 32 + BL, :])
                    nc.vector.tensor_add(c_s3[:], c_s2[:], c_ps[64 : 64 + BL, :])
                    nc.vector.tensor_add(c_sum[:], c_s3[:], c_ps[96 : 96 + BL, :])
                    c_rows = tail.tile([BL, F], BF16, name=f"c_rows{_rep}",
                                        tag="c_rows")
                    nc.vector.tensor_scalar(
                        out=c_rows[:], in0=c_sum[:], scalar1=rz4[:, 0:1],
                        scalar2=None, op0=ALU.mult,
                    )
                    cT = tail.tile([128, NFB, BL], BF16, name=f"cT{_rep}", tag="cT")
                    for fb in range(NFB):
                        tp = ps_tp.tile([128, 128], BF16, tag="tp",
                                        name=f"tpc{_rep}_{fb}")
                        nc.tensor.transpose(
                            tp[:, 0:BL], c_rows[0:BL, 128 * fb : 128 * (fb + 1)],
                            ident[0:BL, 0:BL],
                        )
                        nc.any.tensor_copy(cT[:, fb, :], tp[:, 0:BL])

                    # ---------------- GRU ----------------
                    # re-inject the precomputed gate parts (identity stationary),
                    # then accumulate the context contribution c @ AK; the three
                    # gate blocks run in concurrent PE column-groups
                    g_ps = []
                    for nb in range(3):
                        if nb % 2 == 0:
                            gp = ps_pre.tile([4, TT], F32, tag="pre",
                                             name=f"g_ps{_rep}_{nb}")
                        else:
                            gp = ps_tp.tile([4, TT], F32, tag="tp",
                                            name=f"g_ps{_rep}_{nb}")
                        n0 = nb * TT
                        nc.tensor.matmul(
                            gp[:], ident[0:BL, 0:BL], xh_pre[:, nb, :],
                            start=True, stop=False,
                        )
                        for fb in range(NFB):
                            nc.tensor.matmul(
                                gp[:], cT[:, fb, :], ak_sb[:, fb, n0 : n0 + TT],
                                start=False, stop=(fb == NFB - 1),
                            )
                        g_ps.append(gp[:])

                    def hard_sigmoid(dst, src, nm):
                        tmp = tail.tile([BL, U], F32, name=f"hs_tmp_{_rep}_{nm}",
                                         tag="hs_tmp")
                        nc.vector.tensor_scalar(
                            out=tmp[:], in0=src, scalar1=0.2, scalar2=0.5,
                            op0=ALU.mult, op1=ALU.add,
                        )
                        nc.vector.tensor_scalar(
                            out=dst, in0=tmp[:], scalar1=0.0, scalar2=1.0,
                            op0=ALU.max, op1=ALU.min,
                        )

                    z_sb = tail.tile([BL, U], F32, name=f"z_sb{_rep}", tag="z_sb")
                    r_sb = tail.tile([BL, U], F32, name=f"r_sb{_rep}", tag="r_sb")
                    hard_sigmoid(z_sb[:], g_ps[0], "z")
                    hard_sigmoid(r_sb[:], g_ps[1], "r")

                    rh_bf = tail.tile([BL, U], BF16, name=f"rh_bf{_rep}", tag="rh_bf")
                    nc.vector.tensor_mul(rh_bf[:], r_sb[:], h_f32[:])
                    rhT = tail.tile([128, NUB, BL], BF16, name=f"rhT{_rep}", tag="rhT")
                    for ub in range(NUB):
                        tp = ps_tp.tile([128, 128], BF16, tag="tp",
                                        name=f"tpg{_rep}_{ub}")
                        nc.tensor.transpose(
                            tp[:, 0:BL], rh_bf[0:BL, 128 * ub : 128 * (ub + 1)],
                            ident[0:BL, 0:BL],
                        )
                        nc.any.tensor_copy(rhT[:, ub, :], tp[:, 0:BL])

                    hh_ps = ps_pre.tile([4, TT], F32, tag="pre", name=f"hh_ps{_rep}")
                    for ub in range(NUB):
                        nc.tensor.matmul(
                            hh_ps[:], rhT[:, ub, :], rk_sb[:, ub, 2 * U : 3 * U],
                            start=(ub == 0), stop=(ub == NUB - 1),
                        )

                    xh_sb = tail.tile([BL, U], F32, name=f"xh_sb{_rep}", tag="xh_sb")
                    nc.any.tensor_copy(xh_sb[:], g_ps[2])
                    hh_pre = tail.tile([BL, U], F32, name=f"hh_pre{_rep}", tag="hh_pre")
                    nc.vector.tensor_add(hh_pre[:], xh_sb[:], hh_ps[:])
                    hh = tail.tile([BL, U], F32, name=f"hh{_rep}", tag="hh")
                    nc.scalar.activation(hh[:], hh_pre[:], AF.Tanh)

                    # h_new = hh + z * (h - hh)
                    d_sb = tail.tile([BL, U], F32, name=f"d_sb{_rep}", tag="d_sb")
                    nc.vector.tensor_sub(d_sb[:], h_f32[:], hh[:])
                    zd = tail.tile([BL, U], F32, name=f"zd{_rep}", tag="zd")
                    nc.vector.tensor_mul(zd[:], z_sb[:], d_sb[:])
                    out_sb = tail.tile([BL, U], F32, name=f"out_sb{_rep}", tag="out_sb")
                    nc.vector.tensor_add(out_sb[:], hh[:], zd[:])
                    nc.sync.dma_start(out=d_out, in_=out_sb[:])

                pend_tail[0] = emit_tail


            if pend_tail[0] is not None:
                pend_tail[0]()

    nc.compile()
    return nc


_NC = None


def _get_nc():
    global _NC
    if _NC is None:
        _NC = build()
    return _NC


def kernel(**inputs):
    nc = _get_nc()
    shared = {
        k: np.ascontiguousarray(np.asarray(inputs[k], np.float32))
        for k in (
            "kernel", "recurrent_kernel", "attention_kernel", "Wa", "Ua", "Va",
            "bias", "attention_bias", "Wa_bias", "Ua_bias",
        )
    }
    in_maps = []
    for c in range(NCORES):
        sl = slice(c * BL, (c + 1) * BL)
        m = dict(shared)
        m["x"] = np.ascontiguousarray(np.asarray(inputs["x"], np.float32)[sl])
        m["h"] = np.ascontiguousarray(np.asarray(inputs["h"], np.float32)[sl])
        m["annotations"] = np.ascontiguousarray(
            np.asarray(inputs["annotations"], np.float32)[sl]
        )
        in_maps.append(m)
    res = bass_utils.run_bass_kernel_spmd(nc, in_maps, core_ids=list(range(NCORES)))
    return np.concatenate([r["h_new"] for r in res.results], axis=0)

